# revision 4
# baseline (speedup 1.0000x reference)
"""AFNO1D Trainium2 kernel: 8-way batch-parallel across NeuronCores.

Per core (one batch sample, x [8192, 256] fp32):
  forward : XrT/XiT[c, m] = sum_w x[w,c] * {cos,-sin}(2*pi*w*m/W)   (truncated
            DFT, only M=256 of 4097 rfft modes survive -> plain matmuls)
  MLP     : complex block-diagonal 2-layer MLP (8 blocks of 32x32), exact-erf
            GELU; the DFT's 1/sqrt(W) ortho factors are folded into the MLP
            weights (x enters the forward unscaled).
  inverse : out[w, c] = sum_m alpha_m*(cos*o2r - sin*o2i) + x   (irfft of the
            256-mode spectrum == small matmul; residual added in fp32)

Matmul dtype: bf16 operands, fp32 PSUM accumulation. The AFNO branch is
~2e-4 of the output magnitude (residual dominates), so bf16 branch error is
invisible at the output.
"""
import numpy as np

import concourse.bass as bass
import concourse.mybir as mybir
from concourse import tile
from concourse.vector_clock import ScopedClock
from concourse.bass_utils import run_bass_kernel_spmd

F32 = mybir.dt.float32
BF16 = mybir.dt.bfloat16
NP_BF16 = mybir.dt.np(mybir.dt.bfloat16)

B, W, C = 8, 8192, 256
M, NB, BLK = 256, 8, 32
P = 128
NWC = W // P          # 64 w-chunks of 128
FWG = 8               # fwd: w-chunks per DMA group
NFG = NWC // FWG      # 8 fwd groups
IWG = 16              # inv: w-chunks per DMA group
NIG = NWC // IWG      # 4 inv groups
OWG = 8               # out: w-chunks per store group
SQW = float(np.sqrt(W))

_MAX_SYNC_WAITS = 1


def _split_sync_waits(nc, max_waits=_MAX_SYNC_WAITS):
    """walrus in this container rejects instructions carrying more than one
    sync-wait. Move excess waits onto NoOps inserted just before the
    over-limit instruction on the same engine (bb.instructions is the live
    list shared with the rust module, so in-place insertion works)."""
    n_nop = 0
    for f in nc.m.functions:
        for bb in f.blocks:
            insts = bb.instructions
            idx = 0
            while idx < len(insts):
                inst = insts[idx]
                si = inst.sync_info
                waits = list(si.on_wait) if si and si.on_wait else []
                if len(waits) <= max_waits:
                    idx += 1
                    continue
                keep = waits[-max_waits:]
                rest = waits[:-max_waits]
                inst.sync_info = mybir.SyncInfo(
                    on_wait=keep, on_update=list(si.on_update or [])
                )
                pos = idx
                for i in range(0, len(rest), max_waits):
                    nop = mybir.InstNoOp(
                        name=f"I-waitsplit-{n_nop}",
                        engine=inst.engine,
                        sync_info=mybir.SyncInfo(
                            on_wait=rest[i : i + max_waits], on_update=[]
                        ),
                        bass_nofuse=True,
                    )
                    n_nop += 1
                    nc.register_instruction(nop, overwrite=True)
                    insts.insert(pos, nop)
                    pos += 1
                idx = pos + 1
    return n_nop


def _dft_matrices():
    """fwd_mats [NFG, P, 2, FWG, M] bf16 (k=0 cos, k=1 -sin), raw (no 1/sqrt(W));
    inv_mats [NIG, P, 4, IWG, P] bf16 with alpha folded in
    (k: 0=cos m-chunk0, 1=-sin m0, 2=cos m1, 3=-sin m1)."""
    w = np.arange(W, dtype=np.float64)[:, None]
    m = np.arange(M, dtype=np.float64)[None, :]
    theta = 2.0 * np.pi * ((w * m) % W) / W        # [W, M]
    cf = np.cos(theta)
    sf = -np.sin(theta)
    # [W, M] -> [NFG, FWG, P, M] -> [NFG, P, FWG, M]
    cfr = cf.reshape(NFG, FWG, P, M).transpose(0, 2, 1, 3)
    sfr = sf.reshape(NFG, FWG, P, M).transpose(0, 2, 1, 3)
    fwd = np.stack([cfr, sfr], axis=2).astype(NP_BF16)  # [NFG, P, 2, FWG, M]

    alpha = np.where(np.arange(M) == 0, 1.0, 2.0)[:, None]
    ci = alpha * np.cos(theta.T)     # [M, W]
    si = -alpha * np.sin(theta.T)
    # [M, W] -> per m-chunk [P, NWC, P_w]
    def chunks(a):
        return a.reshape(2, P, NWC, P)  # [mchunk, p, wc, w_lo]
    cic, sic = chunks(ci), chunks(si)
    # k order: cos m0, -sin m0, cos m1, -sin m1
    karr = np.stack([cic[0], sic[0], cic[1], sic[1]], axis=0)  # [4, P, NWC, Pw]
    inv = (
        karr.reshape(4, P, NIG, IWG, P)
        .transpose(2, 1, 0, 3, 4)
        .astype(NP_BF16)
    )  # [NIG, P, 4, IWG, P]
    return np.ascontiguousarray(fwd), np.ascontiguousarray(inv)


def _mlp_arrays(w1, b1, w2, b2):
    """Host-side prep of block-diagonal MLP weights.

    w1t [P, 6, P]: [p, j*3+s, c] = S[j*128+p, j*128+c], S in
      {W1r/sqW, -W1i/sqW, W1i/sqW} for layer-1 lhsT slices.
    w2t [P, 6, P]: same for {W2r/sqW, -W2i/sqW, W2i/sqW} (layer-2 rhs).
    b1t [P, 4] f32: [p, ri*2+j] = b1[ri][j*128+p]  (per-partition gelu bias).
    b2t [1, 512] bf16: [0, ri*256+c] = b2[ri][c]/sqW (bias-init matmul rhs).
    ones [1, P] bf16.
    """
    def bd(blocks):  # [NB, BLK, BLK] -> [C, C]
        out = np.zeros((C, C), np.float64)
        for n in range(NB):
            out[n * BLK:(n + 1) * BLK, n * BLK:(n + 1) * BLK] = blocks[n]
        return out

    w1r = bd(w1[0]) / SQW
    w1i = bd(w1[1]) / SQW
    w2r = bd(w2[0]) / SQW
    w2i = bd(w2[1]) / SQW

    def pack(s0, s1, s2):
        t = np.zeros((P, 6, P), np.float64)
        for j in range(2):
            sl = slice(j * P, (j + 1) * P)
            t[:, j * 3 + 0, :] = s0[sl, sl]
            t[:, j * 3 + 1, :] = s1[sl, sl]
            t[:, j * 3 + 2, :] = s2[sl, sl]
        return t.astype(NP_BF16)

    w1t = pack(w1r, -w1i, w1i)
    w2t = pack(w2r, -w2i, w2i)

    b1t = np.zeros((P, 4), np.float32)
    for ri in range(2):
        flat = np.asarray(b1[ri]).reshape(C)
        for j in range(2):
            b1t[:, ri * 2 + j] = flat[j * P:(j + 1) * P]

    b2t = np.zeros((1, 2 * C), np.float64)
    for ri in range(2):
        b2t[0, ri * C:(ri + 1) * C] = np.asarray(b2[ri]).reshape(C) / SQW
    b2t = b2t.astype(NP_BF16)

    ones = np.ones((1, P), NP_BF16)
    return w1t, w2t, b1t, b2t, ones


def build_nc():
    nc = bass.Bass()
    x_d = nc.declare_dram_parameter("x", [W, C], F32, isOutput=False)
    fwd_d = nc.declare_dram_parameter("fwd_mats", [NFG, P, 2, FWG, M], BF16, isOutput=False)
    inv_d = nc.declare_dram_parameter("inv_mats", [NIG, P, 4, IWG, P], BF16, isOutput=False)
    w1_d = nc.declare_dram_parameter("w1t", [P, 6, P], BF16, isOutput=False)
    w2_d = nc.declare_dram_parameter("w2t", [P, 6, P], BF16, isOutput=False)
    b1_d = nc.declare_dram_parameter("b1t", [P, 4], F32, isOutput=False)
    b2_d = nc.declare_dram_parameter("b2t", [1, 2 * C], BF16, isOutput=False)
    ones_d = nc.declare_dram_parameter("onesv", [1, P], BF16, isOutput=False)
    out_d = nc.declare_dram_parameter("out", [W, C], F32, isOutput=True)

    GELU = mybir.ActivationFunctionType.Gelu
    ADD = mybir.AluOpType.add

    with tile.TileContext(nc) as tc:
        with (
            tc.tile_pool(name="xpool", bufs=1) as xpool,
            tc.tile_pool(name="fwdmat", bufs=2) as fwdpool,
            tc.tile_pool(name="invmat", bufs=2) as invpool,
            tc.tile_pool(name="consts", bufs=1) as constpool,
            tc.tile_pool(name="mlp", bufs=1) as mlppool,
            tc.tile_pool(name="outp", bufs=3) as outpool,
            tc.tile_pool(name="psum", bufs=8, space="PSUM") as pspool,
        ):
            # ---- constants ----
            w1t = constpool.tile([P, 6 * P], BF16)
            w2t = constpool.tile([P, 6 * P], BF16)
            b1t = constpool.tile([P, 4], F32)
            b2t = constpool.tile([1, 2 * C], BF16)
            onest = constpool.tile([1, P], BF16)
            nc.sync.dma_start(w1t.rearrange("p (s c) -> p s c", s=6), w1_d[:])
            nc.sync.dma_start(w2t.rearrange("p (s c) -> p s c", s=6), w2_d[:])
            nc.sync.dma_start(b1t[:], b1_d[:])
            nc.sync.dma_start(b2t[:], b2_d[:])
            nc.sync.dma_start(onest[:], ones_d[:])

            # ---- x load + bf16 convert ----
            x_f32 = xpool.tile([P, NWC * C], F32)   # [p, (wc, c)]
            x_bf = xpool.tile([P, NWC * C], BF16)
            for g in range(NFG):
                src = x_d[g * FWG * P:(g + 1) * FWG * P, :].rearrange(
                    "(t p) c -> p t c", p=P
                )
                dst = x_f32[:, g * FWG * C:(g + 1) * FWG * C].rearrange(
                    "p (t c) -> p t c", t=FWG
                )
                nc.sync.dma_start(dst, src)
                nc.scalar.copy(
                    x_bf[:, g * FWG * C:(g + 1) * FWG * C],
                    x_f32[:, g * FWG * C:(g + 1) * FWG * C],
                )

            # ---- forward DFT: XrT/XiT [c-chunk][128, 256] psum, accumulate over 64 wc
            ps_f = [pspool.tile([P, M], F32, tag="ps", name=f"ps_f{i}") for i in range(4)]
            # order: [xr_c0, xi_c0, xr_c1, xi_c1]
            for g in range(NFG):
                fg = fwdpool.tile([P, 2 * FWG * M], BF16, tag="fwd")
                nc.sync.dma_start(
                    fg.rearrange("p (k t m) -> p k t m", k=2, t=FWG), fwd_d[g]
                )
                for t in range(FWG):
                    wc = g * FWG + t
                    start = wc == 0
                    stop = wc == NWC - 1
                    for cj in range(2):
                        lhsT = x_bf[:, wc * C + cj * P: wc * C + cj * P + P]
                        rhs_c = fg[:, (0 * FWG + t) * M:(0 * FWG + t) * M + M]
                        rhs_s = fg[:, (1 * FWG + t) * M:(1 * FWG + t) * M + M]
                        nc.tensor.matmul(ps_f[2 * cj + 0][:], lhsT, rhs_c, start=start, stop=stop)
                        nc.tensor.matmul(ps_f[2 * cj + 1][:], lhsT, rhs_s, start=start, stop=stop)

            # ---- evacuate forward psums to bf16 sbuf
            xrT = [mlppool.tile([P, M], BF16, name=f"xrT{i}") for i in range(2)]
            xiT = [mlppool.tile([P, M], BF16, name=f"xiT{i}") for i in range(2)]
            for cj in range(2):
                nc.vector.tensor_copy(xrT[cj][:], ps_f[2 * cj + 0][:])
                nc.vector.tensor_copy(xiT[cj][:], ps_f[2 * cj + 1][:])

            # ---- MLP layer 1: o1T[ri][j] [128 oc, 256 m] bf16
            o1T = [[None, None], [None, None]]
            for ri in range(2):
                for j in range(2):
                    ps1 = pspool.tile([P, M], F32, tag="ps")
                    if ri == 0:  # o1r = gelu(W1r^T XrT - W1i^T XiT + b1r)
                        nc.tensor.matmul(ps1[:], w1t[:, (j * 3 + 0) * P:(j * 3 + 1) * P], xrT[j][:], start=True, stop=False)
                        nc.tensor.matmul(ps1[:], w1t[:, (j * 3 + 1) * P:(j * 3 + 2) * P], xiT[j][:], start=False, stop=True)
                    else:        # o1i = gelu(W1i^T XrT + W1r^T XiT + b1i)
                        nc.tensor.matmul(ps1[:], w1t[:, (j * 3 + 2) * P:(j * 3 + 3) * P], xrT[j][:], start=True, stop=False)
                        nc.tensor.matmul(ps1[:], w1t[:, (j * 3 + 0) * P:(j * 3 + 1) * P], xiT[j][:], start=False, stop=True)
                    o1 = mlppool.tile([P, M], BF16, tag=f"o1_{ri}_{j}")
                    nc.scalar.activation(
                        o1[:], ps1[:], GELU,
                        bias=b1t[:, ri * 2 + j: ri * 2 + j + 1],
                    )
                    o1T[ri][j] = o1

            # ---- MLP layer 2: o2[ri][mc] [128 m, 256 oc] -> bf16 sbuf
            # o2r = o1r W2r - o1i W2i + b2r ; o2i = o1i W2r + o1r W2i + b2i
            o2sb = [[None, None], [None, None]]
            for ri in range(2):
                for mc in range(2):
                    ps2 = pspool.tile([P, C], F32, tag="ps")
                    nc.tensor.matmul(
                        ps2[:], onest[:1, :], b2t[:1, ri * C:(ri + 1) * C],
                        start=True, stop=False,
                    )
                    for j in range(2):
                        osl = ps2[:, j * P:(j + 1) * P]
                        last = j == 1
                        if ri == 0:
                            nc.tensor.matmul(osl, o1T[0][j][:, mc * P:(mc + 1) * P], w2t[:, (j * 3 + 0) * P:(j * 3 + 1) * P], start=False, stop=False)
                            nc.tensor.matmul(osl, o1T[1][j][:, mc * P:(mc + 1) * P], w2t[:, (j * 3 + 1) * P:(j * 3 + 2) * P], start=False, stop=last)
                        else:
                            nc.tensor.matmul(osl, o1T[1][j][:, mc * P:(mc + 1) * P], w2t[:, (j * 3 + 0) * P:(j * 3 + 1) * P], start=False, stop=False)
                            nc.tensor.matmul(osl, o1T[0][j][:, mc * P:(mc + 1) * P], w2t[:, (j * 3 + 2) * P:(j * 3 + 3) * P], start=False, stop=last)
                    o2 = mlppool.tile([P, C], BF16, tag=f"o2_{ri}_{mc}")
                    nc.vector.tensor_copy(o2[:], ps2[:])
                    o2sb[ri][mc] = o2

            # rhs order matching inv_mats k: [o2r m0, o2i m0, o2r m1, o2i m1]
            inv_rhs = [o2sb[0][0], o2sb[1][0], o2sb[0][1], o2sb[1][1]]

            # ---- inverse DFT + residual + store
            ot = None
            for gi in range(NIG):
                ig = invpool.tile([P, 4 * IWG * P], BF16, tag="inv")
                nc.sync.dma_start(
                    ig.rearrange("p (k t w) -> p k t w", k=4, t=IWG), inv_d[gi]
                )
                for t in range(IWG):
                    wc = gi * IWG + t
                    og_slot = wc % OWG
                    if og_slot == 0:
                        ot = outpool.tile([P, OWG * C], F32, tag="out")
                    pso = pspool.tile([P, C], F32, tag="ps")
                    for k in range(4):
                        nc.tensor.matmul(
                            pso[:],
                            ig[:, (k * IWG + t) * P:(k * IWG + t) * P + P],
                            inv_rhs[k][:],
                            start=(k == 0), stop=(k == 3),
                        )
                    nc.vector.tensor_tensor(
                        ot[:, og_slot * C:(og_slot + 1) * C],
                        pso[:],
                        x_f32[:, wc * C:(wc + 1) * C],
                        ADD,
                    )
                    if og_slot == OWG - 1:
                        og = wc // OWG
                        dst = out_d[og * OWG * P:(og + 1) * OWG * P, :].rearrange(
                            "(t p) c -> p t c", p=P
                        )
                        nc.sync.dma_start(
                            dst, ot.rearrange("p (t c) -> p t c", t=OWG)
                        )
    _split_sync_waits(nc)
    return nc


_CACHE = {}


def _get_compiled():
    if "nc" not in _CACHE:
        _CACHE["nc"] = build_nc()
        _CACHE["dft"] = _dft_matrices()
    return _CACHE["nc"], _CACHE["dft"]


def kernel(x, w1, b1, w2, b2):
    nc, (fwd_mats, inv_mats) = _get_compiled()
    w1t, w2t, b1t, b2t, ones = _mlp_arrays(
        np.asarray(w1, np.float64), np.asarray(b1, np.float64),
        np.asarray(w2, np.float64), np.asarray(b2, np.float64),
    )
    x = np.asarray(x)
    common = {
        "fwd_mats": fwd_mats, "inv_mats": inv_mats,
        "w1t": w1t, "w2t": w2t, "b1t": b1t, "b2t": b2t, "onesv": ones,
    }
    in_maps = [dict(common, x=np.ascontiguousarray(x[b], np.float32)) for b in range(B)]
    res = run_bass_kernel_spmd(nc, in_maps, core_ids=list(range(B)))
    return np.stack([res.results[i]["out"] for i in range(B)]).astype(np.float32)
